# revision 45
# baseline (speedup 1.0000x reference)
"""MASKGCN Trainium2 kernel: 3-layer masked GCN over B=512 graphs of N=200 nodes.

Strategy
--------
Data-parallel over the batch: 64 graphs per NeuronCore, 8 cores, no collectives.

Math fold (exact up to fp reassociation): the reference network is entirely
LINEAR (no activations), so the whole model collapses:

    mask = (E + E^T)/2 + I
    A_g  = sigmoid(adj_g) * mask        (adj binary -> sigmoid(adj) = c*(adj+s))
    out_g = (1/200) * 1^T A_g^3 F_g (W0 W1 W2 pw) + pb

With Wf = W0@W1@W2@pw/200 host-folded ([200,2]), evaluate RIGHT-TO-LEFT
(the "Y-chain"), folding the outermost 1^T*A into a colsum:

    Y0 = F Wf          [200,2] per graph
    Y1 = A Y0          [200,2]
    Y2 = A Y1          [200,2]
    out = r0^T Y2 + pb where r0 = colsum(A)

Device layout per core (bf16, graph-major, TRANSPOSED):
    at_all [m, g*200+n] = A_g[n, m]   built by DVE from adj^T and c*mask
                                      (mask is symmetric so the same tiles
                                      serve both orientations); the build's
                                      fused accum_out yields r0 = colsum(A)
                                      as ready-made columns — a whole PE pass
                                      for free
    fT_all [f, g*200+n] = F_g[n, f]
Each Y-matvec = 4 PE matmuls (2 K-tiles x 2 M-tiles) with the At/Ft tile as
the stationary and a 2-wide Y column pair as the moving rhs; outputs pack as
column pairs in PSUM [n_tile, 2*64] banks (one evacuation per pass). The
final r0^T Y2 uses the tiny r0 column as the stationary, packing all graphs
into one PSUM row. PE weight-load volume: 3 full passes instead of the
4 an Aᵀ-chain needs.
"""

import os
import sys
import numpy as np

if "concourse" not in sys.modules:
    try:
        import concourse  # noqa: F401
    except ImportError:
        for _p in ("/opt/trn_rl_repo", "/root/.axon_site/_ro/trn_rl_repo"):
            if os.path.isdir(_p) and _p not in sys.path:
                sys.path.append(_p)

import ml_dtypes

B, N, IN_C, HID, OUT_C, N_VARS = 512, 200, 200, 256, 256, 2
N_CORES = 8
BPC = B // N_CORES  # graphs per core
P0 = 128
P1 = N - P0  # 72

# sigmoid(adj) = C_SIG * (adj + S_SIG) for adj in {0, 1}
C_SIG = float(1.0 / (1.0 + np.exp(-1.0)) - 0.5)
S_SIG = float(0.5 / C_SIG)

BF16 = ml_dtypes.bfloat16

_BUILD_CACHE = {}


def _build_nc(bpc, reps=1):
    """Per-core Bass program (SPMD: identical on all cores).

    reps>1 wraps the batch in a hardware For_i — benchmarking only."""
    import concourse.bacc as bacc
    import concourse.mybir as mybir
    import concourse.tile as tile
    from contextlib import ExitStack

    f32 = mybir.dt.float32
    bf16 = mybir.dt.bfloat16
    ADD = mybir.AluOpType.add
    MULT = mybir.AluOpType.mult

    W = bpc * N   # 12800 free columns for the big graph-major tiles
    W2c = bpc * N_VARS  # 128 columns in the Y banks

    nc = bacc.Bacc(None, target_bir_lowering=False)
    adjp = nc.declare_dram_parameter("adjp", [N, W], mybir.dt.uint8, isOutput=False)
    fp_ = nc.declare_dram_parameter("fp", [N, W], bf16, isOutput=False)
    maskp = nc.declare_dram_parameter("maskp", [N, N], bf16, isOutput=False)
    wfp = nc.declare_dram_parameter("wfp", [N, N_VARS], bf16, isOutput=False)
    onesp = nc.declare_dram_parameter("onesp", [P0, 1], f32, isOutput=False)
    out = nc.declare_dram_parameter("out", [1, W2c], f32, isOutput=True)

    with tile.TileContext(nc) as tc, ExitStack() as ctx:
        consts = ctx.enter_context(tc.tile_pool(name="consts", bufs=1))
        big = ctx.enter_context(tc.tile_pool(name="big", bufs=1))
        psy = ctx.enter_context(tc.tile_pool(name="psy", bufs=3, space="PSUM"))
        psout = ctx.enter_context(tc.tile_pool(name="psout", bufs=1, space="PSUM"))

        # ---- constants ----
        mk_a = consts.tile([P0, N], bf16, tag="mk_a")
        mk_b = consts.tile([P1, N], bf16, tag="mk_b")
        wf_a = consts.tile([P0, N_VARS], bf16, tag="wf_a")
        wf_b = consts.tile([P1, N_VARS], bf16, tag="wf_b")
        ones_t = consts.tile([P0, 1], f32, tag="ones_t")
        nc.sync.dma_start(mk_a[:], maskp[0:P0, :])
        nc.sync.dma_start(mk_b[:], maskp[P0:N, :])
        nc.sync.dma_start(wf_a[:], wfp[0:P0, :])
        nc.sync.dma_start(wf_b[:], wfp[P0:N, :])
        nc.sync.dma_start(ones_t[:], onesp[:, :])

        # ---- big graph-major tiles (transposed layouts) ----
        adj_a = big.tile([P0, W], mybir.dt.uint8, tag="adj_a")
        adj_b = big.tile([P1, W], mybir.dt.uint8, tag="adj_b")
        f_a = big.tile([P0, W], bf16, tag="f_a")
        f_b = big.tile([P1, W], bf16, tag="f_b")
        at_a = big.tile([P0, W], bf16, tag="at_a")
        at_b = big.tile([P1, W], bf16, tag="at_b")

        # r0 (colsum A) accumulators and bf16 copies
        r0f_a = big.tile([P0, bpc], f32, tag="r0f_a")
        r0f_b = big.tile([P1, bpc], f32, tag="r0f_b")
        r0b_a = big.tile([P0, bpc], bf16, tag="r0b_a")
        r0b_b = big.tile([P1, bpc], bf16, tag="r0b_b")

        # Y column-pair banks [n_tile, g*2+j]
        y0a = big.tile([P0, W2c], bf16, tag="y0a")
        y0b = big.tile([P1, W2c], bf16, tag="y0b")
        y1a = big.tile([P0, W2c], bf16, tag="y1a")
        y1b = big.tile([P1, W2c], bf16, tag="y1b")
        y2a = big.tile([P0, W2c], bf16, tag="y2a")
        y2b = big.tile([P1, W2c], bf16, tag="y2b")
        prod_a = big.tile([P0, W2c], f32, tag="prod_a")
        prod_b = big.tile([P1, W2c], f32, tag="prod_b")
        out_sb = big.tile([1, W2c], f32, tag="out_sb")

        def emit_batch():
            # ---- DMA (chunked for overlap) ----
            BCH = 16
            BCW = BCH * N
            for c in range(bpc // BCH):
                cs, ce = c * BCW, (c + 1) * BCW
                nc.sync.dma_start(adj_a[:, cs:ce], adjp[0:P0, cs:ce])
                nc.sync.dma_start(adj_b[:, cs:ce], adjp[P0:N, cs:ce])
                nc.sync.dma_start(f_a[:, cs:ce], fp_[0:P0, cs:ce])
                nc.sync.dma_start(f_b[:, cs:ce], fp_[P0:N, cs:ce])

            # ---- build At per graph (DVE), colsum fused via accum_out ----
            # At = (adjT + s) * (c*mask); accum over n gives r0 = colsum(A)
            for g in range(bpc):
                gs = g * N
                nc.vector.scalar_tensor_tensor(
                    at_a[:, gs:gs + N], adj_a[:, gs:gs + N], S_SIG, mk_a[:],
                    op0=ADD, op1=MULT, accum_out=r0f_a[:, g:g + 1],
                )
                nc.vector.scalar_tensor_tensor(
                    at_b[:, gs:gs + N], adj_b[:, gs:gs + N], S_SIG, mk_b[:],
                    op0=ADD, op1=MULT, accum_out=r0f_b[:, g:g + 1],
                )
            nc.vector.tensor_copy(r0b_a[:], r0f_a[:])
            nc.vector.tensor_copy(r0b_b[:], r0f_b[:])

            def emit_y_pass(lhs_pair, rhs_of_g, dst):
                """One Y pass: out cols [n_tile, 2] per graph, all 64 graphs
                into one PSUM tile pair. Evacuation is split in halves so the
                first half lands while the PE works the second half — the
                next pass's first matmuls then start without waiting."""
                ps_a = psy.tile([P0, W2c], f32, tag="pya")
                ps_b = psy.tile([P1, W2c], f32, tag="pyb")
                EQ = bpc // 8  # evac granularity (graphs)
                for g in range(bpc):
                    if g % EQ == 0 and g > 0:
                        e0, e1 = (g - EQ) * N_VARS, g * N_VARS
                        nc.scalar.copy(dst[0][:, e0:e1], ps_a[:, e0:e1])
                        nc.scalar.copy(dst[1][:, e0:e1], ps_b[:, e0:e1])
                    gs = g * N
                    g2 = g * N_VARS
                    rh_a, rh_b = rhs_of_g(g)
                    # out n-tile [0, 128)
                    nc.tensor.matmul(
                        ps_a[:, g2:g2 + N_VARS],
                        lhs_pair[0][:, gs:gs + P0], rh_a,
                        start=True, stop=False,
                    )
                    nc.tensor.matmul(
                        ps_a[:, g2:g2 + N_VARS],
                        lhs_pair[1][:, gs:gs + P0], rh_b,
                        start=False, stop=True,
                    )
                    # out n-tile [128, 200)
                    nc.tensor.matmul(
                        ps_b[:, g2:g2 + N_VARS],
                        lhs_pair[0][:, gs + P0:gs + N], rh_a,
                        start=True, stop=False,
                    )
                    nc.tensor.matmul(
                        ps_b[:, g2:g2 + N_VARS],
                        lhs_pair[1][:, gs + P0:gs + N], rh_b,
                        start=False, stop=True,
                    )
                e0 = (bpc - EQ) * N_VARS
                nc.scalar.copy(dst[0][:, e0:], ps_a[:, e0:])
                nc.scalar.copy(dst[1][:, e0:], ps_b[:, e0:])

            # Y0 = F @ Wf   (stationary fT tiles, moving Wf column pair)
            emit_y_pass(
                (f_a, f_b),
                lambda g: (wf_a[:], wf_b[:]),
                (y0a, y0b),
            )
            # Y1 = A @ Y0   (stationary At tiles, moving Y0 column pair)
            emit_y_pass(
                (at_a, at_b),
                lambda g: (y0a[:, g * N_VARS:(g + 1) * N_VARS],
                           y0b[:, g * N_VARS:(g + 1) * N_VARS]),
                (y1a, y1b),
            )
            # Y2 = A @ Y1
            emit_y_pass(
                (at_a, at_b),
                lambda g: (y1a[:, g * N_VARS:(g + 1) * N_VARS],
                           y1b[:, g * N_VARS:(g + 1) * N_VARS]),
                (y2a, y2b),
            )

            # ---- final: out[g,:] = r0_g^T Y2_g for all graphs at once ----
            # DVE: prod[m, (g,j)] = Y2[m, (g,j)] * r0[m, g]  (free-dim bcast)
            # PE: two ones-matmuls partition-reduce prod over m.
            nc.vector.scalar_tensor_tensor(
                prod_a[:].rearrange("p (g j) -> p g j", g=bpc),
                y2a[:].rearrange("p (g j) -> p g j", g=bpc),
                1.0,
                r0b_a[:].unsqueeze(2).broadcast_to((P0, bpc, N_VARS)),
                op0=MULT, op1=MULT,
            )
            nc.vector.scalar_tensor_tensor(
                prod_b[:].rearrange("p (g j) -> p g j", g=bpc),
                y2b[:].rearrange("p (g j) -> p g j", g=bpc),
                1.0,
                r0b_b[:].unsqueeze(2).broadcast_to((P1, bpc, N_VARS)),
                op0=MULT, op1=MULT,
            )
            po = psout.tile([1, W2c], f32, tag="po")
            nc.tensor.matmul(
                po[:], ones_t[0:P0, :], prod_a[:], start=True, stop=False)
            nc.tensor.matmul(
                po[:], ones_t[0:P1, :], prod_b[:], start=False, stop=True)
            nc.vector.tensor_copy(out_sb[:], po[:])

        if reps > 1:
            with tc.For_i(0, reps, 1):
                emit_batch()
        else:
            emit_batch()

        nc.sync.dma_start(out[:], out_sb[:])

    nc.compile()
    return nc


def _host_prep(adj, features, raw_edge_weight, W0, W1, W2, pw, pb):
    """Host-side weight folding + per-core graph-major transposed bf16 shards."""
    mask = ((raw_edge_weight.astype(np.float64)
             + raw_edge_weight.astype(np.float64).T) * 0.5
            + np.eye(N, dtype=np.float64))
    maskc = (C_SIG * mask).astype(BF16)
    wf = (W0.astype(np.float64) @ W1.astype(np.float64)
          @ W2.astype(np.float64) @ pw.astype(np.float64) / float(N)
          ).astype(BF16)
    in_maps = []
    for c in range(N_CORES):
        sl = slice(c * BPC, (c + 1) * BPC)
        # [g, n, x] -> [x, g*200 + n]  (transposed per graph)
        a_mn = np.ascontiguousarray(
            adj[sl].transpose(2, 0, 1).reshape(N, BPC * N)).astype(np.uint8)
        f_fn = np.ascontiguousarray(
            features[sl].transpose(2, 0, 1).reshape(IN_C, BPC * N)).astype(BF16)
        in_maps.append({
            "adjp": a_mn,
            "fp": f_fn,
            "maskp": maskc,
            "wfp": wf,
            "onesp": np.ones((P0, 1), dtype=np.float32),
        })
    return in_maps


def kernel(adj, features, raw_edge_weight, W0, W1, W2, pw, pb, _trace=False):
    from concourse.bass_utils import run_bass_kernel_spmd

    adj = np.asarray(adj, dtype=np.float32)
    features = np.asarray(features, dtype=np.float32)
    raw_edge_weight = np.asarray(raw_edge_weight, dtype=np.float32)
    W0 = np.asarray(W0, dtype=np.float32)
    W1 = np.asarray(W1, dtype=np.float32)
    W2 = np.asarray(W2, dtype=np.float32)
    pw = np.asarray(pw, dtype=np.float32)
    pb = np.asarray(pb, dtype=np.float32)

    if "nc" not in _BUILD_CACHE:
        _BUILD_CACHE["nc"] = _build_nc(BPC)
    nc = _BUILD_CACHE["nc"]

    in_maps = _host_prep(adj, features, raw_edge_weight, W0, W1, W2, pw, pb)
    res = run_bass_kernel_spmd(
        nc, in_maps, core_ids=list(range(N_CORES)), trace=bool(_trace)
    )
    out = np.concatenate(
        [res.results[c]["out"].reshape(BPC, N_VARS) for c in range(N_CORES)],
        axis=0,
    )
    out = out + pb[None, :].astype(np.float32)
    if _trace:
        return out, res
    return out


# revision 46
# speedup vs baseline: 1.0392x; 1.0392x over previous
"""MASKGCN Trainium2 kernel: 3-layer masked GCN over B=512 graphs of N=200 nodes.

Strategy
--------
Data-parallel over the batch: 64 graphs per NeuronCore, 8 cores, no collectives.

Math fold (exact up to fp reassociation): the reference network is entirely
LINEAR (no activations), so the whole model collapses:

    mask = (E + E^T)/2 + I
    A_g  = sigmoid(adj_g) * mask        (adj binary -> sigmoid(adj) = c*(adj+s))
    out_g = (1/200) * 1^T A_g^3 F_g (W0 W1 W2 pw) + pb

With Wf = W0@W1@W2@pw/200 host-folded ([200,2]), evaluate RIGHT-TO-LEFT
(the "Y-chain"), folding the outermost 1^T*A into a colsum:

    Y0 = F Wf          [200,2] per graph
    Y1 = A Y0          [200,2]
    Y2 = A Y1          [200,2]
    out = r0^T Y2 + pb where r0 = colsum(A)

Device layout per core (bf16, graph-major, TRANSPOSED):
    at_all [m, g*200+n] = A_g[n, m]   built by DVE from adj^T and c*mask
                                      (mask is symmetric so the same tiles
                                      serve both orientations); the build's
                                      fused accum_out yields r0 = colsum(A)
                                      as ready-made columns — a whole PE pass
                                      for free
    fT_all [f, g*200+n] = F_g[n, f]
Each Y-matvec = 4 PE matmuls (2 K-tiles x 2 M-tiles) with the At/Ft tile as
the stationary and a 2-wide Y column pair as the moving rhs; outputs pack as
column pairs in PSUM [n_tile, 2*64] banks (one evacuation per pass). The
final r0^T Y2 uses the tiny r0 column as the stationary, packing all graphs
into one PSUM row. PE weight-load volume: 3 full passes instead of the
4 an Aᵀ-chain needs.
"""

import os
import sys
import numpy as np

if "concourse" not in sys.modules:
    try:
        import concourse  # noqa: F401
    except ImportError:
        for _p in ("/opt/trn_rl_repo", "/root/.axon_site/_ro/trn_rl_repo"):
            if os.path.isdir(_p) and _p not in sys.path:
                sys.path.append(_p)

import ml_dtypes

B, N, IN_C, HID, OUT_C, N_VARS = 512, 200, 200, 256, 256, 2
N_CORES = 8
BPC = B // N_CORES  # graphs per core
P0 = 128
P1 = N - P0  # 72

# sigmoid(adj) = C_SIG * (adj + S_SIG) for adj in {0, 1}
C_SIG = float(1.0 / (1.0 + np.exp(-1.0)) - 0.5)
S_SIG = float(0.5 / C_SIG)

BF16 = ml_dtypes.bfloat16

_BUILD_CACHE = {}


def _build_nc(bpc, reps=1):
    """Per-core Bass program (SPMD: identical on all cores).

    reps>1 wraps the batch in a hardware For_i — benchmarking only."""
    import concourse.bacc as bacc
    import concourse.mybir as mybir
    import concourse.tile as tile
    from contextlib import ExitStack

    f32 = mybir.dt.float32
    bf16 = mybir.dt.bfloat16
    ADD = mybir.AluOpType.add
    MULT = mybir.AluOpType.mult

    W = bpc * N   # 12800 free columns for the big graph-major tiles
    W2c = bpc * N_VARS  # 128 columns in the Y banks

    nc = bacc.Bacc(None, target_bir_lowering=False)
    adjp = nc.declare_dram_parameter("adjp", [N, W], mybir.dt.uint8, isOutput=False)
    fp_ = nc.declare_dram_parameter("fp", [N, W], bf16, isOutput=False)
    maskp = nc.declare_dram_parameter("maskp", [N, N], bf16, isOutput=False)
    wfp = nc.declare_dram_parameter("wfp", [N, N_VARS], bf16, isOutput=False)
    onesp = nc.declare_dram_parameter("onesp", [P0, 1], f32, isOutput=False)
    out = nc.declare_dram_parameter("out", [1, W2c], f32, isOutput=True)

    with tile.TileContext(nc) as tc, ExitStack() as ctx:
        consts = ctx.enter_context(tc.tile_pool(name="consts", bufs=1))
        big = ctx.enter_context(tc.tile_pool(name="big", bufs=1))
        psy = ctx.enter_context(tc.tile_pool(name="psy", bufs=3, space="PSUM"))
        psout = ctx.enter_context(tc.tile_pool(name="psout", bufs=1, space="PSUM"))

        # ---- constants ----
        mk_a = consts.tile([P0, N], bf16, tag="mk_a")
        mk_b = consts.tile([P1, N], bf16, tag="mk_b")
        wf_a = consts.tile([P0, N_VARS], bf16, tag="wf_a")
        wf_b = consts.tile([P1, N_VARS], bf16, tag="wf_b")
        ones_t = consts.tile([P0, 1], f32, tag="ones_t")
        nc.sync.dma_start(mk_a[:], maskp[0:P0, :])
        nc.sync.dma_start(mk_b[:], maskp[P0:N, :])
        nc.sync.dma_start(wf_a[:], wfp[0:P0, :])
        nc.sync.dma_start(wf_b[:], wfp[P0:N, :])
        nc.sync.dma_start(ones_t[:], onesp[:, :])

        # ---- big graph-major tiles (transposed layouts) ----
        adj_a = big.tile([P0, W], mybir.dt.uint8, tag="adj_a")
        adj_b = big.tile([P1, W], mybir.dt.uint8, tag="adj_b")
        f_a = big.tile([P0, W], bf16, tag="f_a")
        f_b = big.tile([P1, W], bf16, tag="f_b")
        at_a = big.tile([P0, W], bf16, tag="at_a")
        at_b = big.tile([P1, W], bf16, tag="at_b")

        # r0 (colsum A) accumulators and bf16 copies
        r0f_a = big.tile([P0, bpc], f32, tag="r0f_a")
        r0f_b = big.tile([P1, bpc], f32, tag="r0f_b")
        r0b_a = big.tile([P0, bpc], bf16, tag="r0b_a")
        r0b_b = big.tile([P1, bpc], bf16, tag="r0b_b")

        # Y column-pair banks [n_tile, g*2+j]
        y0a = big.tile([P0, W2c], bf16, tag="y0a")
        y0b = big.tile([P1, W2c], bf16, tag="y0b")
        y1a = big.tile([P0, W2c], bf16, tag="y1a")
        y1b = big.tile([P1, W2c], bf16, tag="y1b")
        y2a = big.tile([P0, W2c], bf16, tag="y2a")
        y2b = big.tile([P1, W2c], bf16, tag="y2b")
        prod_a = big.tile([P0, W2c], f32, tag="prod_a")
        prod_b = big.tile([P1, W2c], f32, tag="prod_b")
        out_sb = big.tile([1, W2c], f32, tag="out_sb")

        def emit_batch():
            # ---- DMA (chunked for overlap) ----
            BCH = 16
            BCW = BCH * N
            for c in range(bpc // BCH):
                cs, ce = c * BCW, (c + 1) * BCW
                nc.sync.dma_start(adj_a[:, cs:ce], adjp[0:P0, cs:ce])
                nc.sync.dma_start(adj_b[:, cs:ce], adjp[P0:N, cs:ce])
                nc.sync.dma_start(f_a[:, cs:ce], fp_[0:P0, cs:ce])
                nc.sync.dma_start(f_b[:, cs:ce], fp_[P0:N, cs:ce])

            # ---- build At per graph (DVE), colsum fused via accum_out ----
            # At = (adjT + s) * (c*mask); accum over n gives r0 = colsum(A)
            for g in range(bpc):
                gs = g * N
                nc.vector.scalar_tensor_tensor(
                    at_a[:, gs:gs + N], adj_a[:, gs:gs + N], S_SIG, mk_a[:],
                    op0=ADD, op1=MULT, accum_out=r0f_a[:, g:g + 1],
                )
                nc.vector.scalar_tensor_tensor(
                    at_b[:, gs:gs + N], adj_b[:, gs:gs + N], S_SIG, mk_b[:],
                    op0=ADD, op1=MULT, accum_out=r0f_b[:, g:g + 1],
                )
            nc.vector.tensor_copy(r0b_a[:], r0f_a[:])
            nc.vector.tensor_copy(r0b_b[:], r0f_b[:])

            def emit_y_pass(lhs_pair, rhs_of_g, dst):
                """One Y pass: out cols [n_tile, 2] per graph, all 64 graphs
                into one PSUM tile pair. Evacuation is split in halves so the
                first half lands while the PE works the second half — the
                next pass's first matmuls then start without waiting."""
                ps_a = psy.tile([P0, W2c], f32, tag="pya")
                ps_b = psy.tile([P1, W2c], f32, tag="pyb")
                EQ = bpc // 4  # evac granularity (graphs)
                for g in range(bpc):
                    if g % EQ == 0 and g > 0:
                        e0, e1 = (g - EQ) * N_VARS, g * N_VARS
                        nc.scalar.copy(dst[0][:, e0:e1], ps_a[:, e0:e1])
                        nc.scalar.copy(dst[1][:, e0:e1], ps_b[:, e0:e1])
                    gs = g * N
                    g2 = g * N_VARS
                    rh_a, rh_b = rhs_of_g(g)
                    # out n-tile [0, 128)
                    nc.tensor.matmul(
                        ps_a[:, g2:g2 + N_VARS],
                        lhs_pair[0][:, gs:gs + P0], rh_a,
                        start=True, stop=False,
                    )
                    nc.tensor.matmul(
                        ps_a[:, g2:g2 + N_VARS],
                        lhs_pair[1][:, gs:gs + P0], rh_b,
                        start=False, stop=True,
                    )
                    # out n-tile [128, 200)
                    nc.tensor.matmul(
                        ps_b[:, g2:g2 + N_VARS],
                        lhs_pair[0][:, gs + P0:gs + N], rh_a,
                        start=True, stop=False,
                    )
                    nc.tensor.matmul(
                        ps_b[:, g2:g2 + N_VARS],
                        lhs_pair[1][:, gs + P0:gs + N], rh_b,
                        start=False, stop=True,
                    )
                e0 = (bpc - EQ) * N_VARS
                nc.scalar.copy(dst[0][:, e0:], ps_a[:, e0:])
                nc.scalar.copy(dst[1][:, e0:], ps_b[:, e0:])

            # Y0 = F @ Wf   (stationary fT tiles, moving Wf column pair)
            emit_y_pass(
                (f_a, f_b),
                lambda g: (wf_a[:], wf_b[:]),
                (y0a, y0b),
            )
            # Y1 = A @ Y0   (stationary At tiles, moving Y0 column pair)
            emit_y_pass(
                (at_a, at_b),
                lambda g: (y0a[:, g * N_VARS:(g + 1) * N_VARS],
                           y0b[:, g * N_VARS:(g + 1) * N_VARS]),
                (y1a, y1b),
            )
            # Y2 = A @ Y1
            emit_y_pass(
                (at_a, at_b),
                lambda g: (y1a[:, g * N_VARS:(g + 1) * N_VARS],
                           y1b[:, g * N_VARS:(g + 1) * N_VARS]),
                (y2a, y2b),
            )

            # ---- final: out[g,:] = r0_g^T Y2_g for all graphs at once ----
            # DVE: prod[m, (g,j)] = Y2[m, (g,j)] * r0[m, g]  (free-dim bcast)
            # PE: two ones-matmuls partition-reduce prod over m.
            nc.vector.scalar_tensor_tensor(
                prod_a[:].rearrange("p (g j) -> p g j", g=bpc),
                y2a[:].rearrange("p (g j) -> p g j", g=bpc),
                1.0,
                r0b_a[:].unsqueeze(2).broadcast_to((P0, bpc, N_VARS)),
                op0=MULT, op1=MULT,
            )
            nc.vector.scalar_tensor_tensor(
                prod_b[:].rearrange("p (g j) -> p g j", g=bpc),
                y2b[:].rearrange("p (g j) -> p g j", g=bpc),
                1.0,
                r0b_b[:].unsqueeze(2).broadcast_to((P1, bpc, N_VARS)),
                op0=MULT, op1=MULT,
            )
            po = psout.tile([1, W2c], f32, tag="po")
            nc.tensor.matmul(
                po[:], ones_t[0:P0, :], prod_a[:], start=True, stop=False)
            nc.tensor.matmul(
                po[:], ones_t[0:P1, :], prod_b[:], start=False, stop=True)
            nc.vector.tensor_copy(out_sb[:], po[:])

        if reps > 1:
            with tc.For_i(0, reps, 1):
                emit_batch()
        else:
            emit_batch()

        nc.sync.dma_start(out[:], out_sb[:])

    nc.compile()
    return nc


def _host_prep(adj, features, raw_edge_weight, W0, W1, W2, pw, pb):
    """Host-side weight folding + per-core graph-major transposed bf16 shards."""
    mask = ((raw_edge_weight.astype(np.float64)
             + raw_edge_weight.astype(np.float64).T) * 0.5
            + np.eye(N, dtype=np.float64))
    maskc = (C_SIG * mask).astype(BF16)
    wf = (W0.astype(np.float64) @ W1.astype(np.float64)
          @ W2.astype(np.float64) @ pw.astype(np.float64) / float(N)
          ).astype(BF16)
    in_maps = []
    for c in range(N_CORES):
        sl = slice(c * BPC, (c + 1) * BPC)
        # [g, n, x] -> [x, g*200 + n]  (transposed per graph)
        a_mn = np.ascontiguousarray(
            adj[sl].transpose(2, 0, 1).reshape(N, BPC * N)).astype(np.uint8)
        f_fn = np.ascontiguousarray(
            features[sl].transpose(2, 0, 1).reshape(IN_C, BPC * N)).astype(BF16)
        in_maps.append({
            "adjp": a_mn,
            "fp": f_fn,
            "maskp": maskc,
            "wfp": wf,
            "onesp": np.ones((P0, 1), dtype=np.float32),
        })
    return in_maps


def kernel(adj, features, raw_edge_weight, W0, W1, W2, pw, pb, _trace=False):
    from concourse.bass_utils import run_bass_kernel_spmd

    adj = np.asarray(adj, dtype=np.float32)
    features = np.asarray(features, dtype=np.float32)
    raw_edge_weight = np.asarray(raw_edge_weight, dtype=np.float32)
    W0 = np.asarray(W0, dtype=np.float32)
    W1 = np.asarray(W1, dtype=np.float32)
    W2 = np.asarray(W2, dtype=np.float32)
    pw = np.asarray(pw, dtype=np.float32)
    pb = np.asarray(pb, dtype=np.float32)

    if "nc" not in _BUILD_CACHE:
        _BUILD_CACHE["nc"] = _build_nc(BPC)
    nc = _BUILD_CACHE["nc"]

    in_maps = _host_prep(adj, features, raw_edge_weight, W0, W1, W2, pw, pb)
    res = run_bass_kernel_spmd(
        nc, in_maps, core_ids=list(range(N_CORES)), trace=bool(_trace)
    )
    out = np.concatenate(
        [res.results[c]["out"].reshape(BPC, N_VARS) for c in range(N_CORES)],
        axis=0,
    )
    out = out + pb[None, :].astype(np.float32)
    if _trace:
        return out, res
    return out
